# revision 36
# baseline (speedup 1.0000x reference)
"""Trainium2 (8 NeuronCores) kernel for nn_BlockModel_9758165696627.

GNN message passing: 2 residual blocks of
  gather(nbr) + gather(self) + add_info -> MLP(relu) -> segment_max -> @Wo + residual
then a final 129 -> 64 -> 1 MLP.

Strategy (node/segment sharding, v2):
  N = 15872 nodes, E = 253952 edges. Each of the 8 cores owns N/8 = 1984
  contiguous segments; since self_indexes are sorted, each core owns a
  contiguous slice of edges and the segment max is entirely local.

  First-layer refactor: x @ W1 = A[nbr] + B[self] + w1r*add_info where
  A = interp @ W1[:129], B = interp @ W1[129:258] + b1, w1r = W1[258].

  Block 0's A/B tables depend only on kernel inputs, so the host computes
  them (A0 replicated to every core's DRAM, B0' direct to SBUF) - no
  startup AllGather.  Block 1's tables are built on device from the
  updated features and AllGathered as before.

  Per-edge A rows are fetched with NON-transpose dma_gather (256B
  contiguous writes), pipelined with prepare_only+trigger_dma across all
  4 SWDGE queues - concurrent queues give ~2-4x aggregate gather
  bandwidth, and non-transpose mode is safe across queues (transpose
  xbar sprays corrupt each other; verified on HW).  The edge-major tiles
  are transposed to feature-major on the idle PE via identity matmuls
  into PSUM, with the w1r*add_info outer product accumulated into the
  same PSUM bank.

  h2's relu+bias commute past the ragged segment max (relu monotonic,
  bias per-feature), so the max reduces W2's PSUM output directly and
  relu(x+b2) is applied to the pooled (N-sized) result only.

  Edges are laid out region-major, padded so no segment straddles a
  512-slot PSUM-bank boundary.
"""

import numpy as np
import ml_dtypes

BF16 = ml_dtypes.bfloat16

N = 15872
D = 129
H = 128
NCORES = 8
NPC = N // NCORES  # 1984 nodes per core
TILE_SLOTS = 2048  # edge slots per gather tile (4 x 512)
NQ = 4             # SWDGE queues


# ---------------------------------------------------------------------------
# host-side preprocessing
# ---------------------------------------------------------------------------

def _arith_runs(nodes):
    """Split a sorted node list into (start, stride, count) arithmetic runs."""
    runs = []
    i = 0
    n = len(nodes)
    while i < n:
        if i + 1 == n:
            runs.append((int(nodes[i]), 1, 1))
            break
        stride = int(nodes[i + 1] - nodes[i])
        j = i + 1
        while j + 1 < n and nodes[j + 1] - nodes[j] == stride:
            j += 1
        runs.append((int(nodes[i]), stride, j - i + 1))
        i = j + 1
    return runs


def _layout(sizes_core):
    """Region-major tiles with 512-aligned segments.

    Each tile is dict(width, pieces); pieces are
    (size, nseg, node0, node_stride, slot_off), all segments of a piece
    inside one 512-slot block (padding inserted at block boundaries).
    """
    order = []
    for s in np.unique(sizes_core):
        nodes = np.where(sizes_core == s)[0]
        for (n0, st, cnt) in _arith_runs(nodes):
            order.append((int(s), n0, st, cnt))

    tiles = []
    cur = []
    cur_slots = 0

    def cap():
        # first 4 tiles stay small (512 slots): their transfers land fast,
        # so compute starts early at block start and right after the AG
        return 512 if len(tiles) < 4 else TILE_SLOTS

    def flush():
        nonlocal cur, cur_slots
        if cur:
            w = -(-cur_slots // 128) * 128
            tiles.append(dict(width=w, pieces=cur))
            cur, cur_slots = [], 0

    for (s, n0, st, cnt) in order:
        done = 0
        while done < cnt:
            room = (512 - cur_slots % 512) // s
            if room == 0:
                cur_slots = -(-cur_slots // 512) * 512  # pad to 512
                if cur_slots >= cap():
                    flush()
                room = 512 // s
            take = min(cnt - done, room,
                       (cap() - cur_slots) // s if s else 0)
            cur.append((s, take, n0 + done * st, st, cur_slots))
            cur_slots += take * s
            done += take
            if cur_slots >= cap():
                flush()
    flush()
    tiles.sort(key=lambda t: t["width"])
    return tiles


def _preprocess(sizes, nbr, a):
    csum = np.zeros(N + 1, np.int64)
    np.cumsum(sizes, out=csum[1:])

    sizes0 = sizes[:NPC]
    uniform = all(
        np.array_equal(sizes[c * NPC:(c + 1) * NPC], sizes0)
        for c in range(NCORES)
    )
    assert uniform, "per-core segment-size patterns differ; unsupported"
    tiles = _layout(sizes0)
    SP = sum(t["width"] for t in tiles)

    def wrap16(idx):
        n = len(idx)
        assert n % 16 == 0
        w = idx.astype(np.int16).reshape(n // 16, 16).T
        return np.ascontiguousarray(np.tile(w, (8, 1)))

    cores = []
    for c in range(NCORES):
        idx_stream = np.zeros(SP, np.int64)
        a_stream = np.zeros(SP, np.float32)
        off = 0
        for t in tiles:
            for (s, nseg, node0, nstride, slot_off) in t["pieces"]:
                for k in range(nseg):
                    g = c * NPC + node0 + k * nstride
                    e0, e1 = csum[g], csum[g + 1]
                    assert e1 - e0 == s
                    p = off + slot_off + k * s
                    idx_stream[p:p + s] = nbr[e0:e1]
                    a_stream[p:p + s] = a[e0:e1]
            off += t["width"]
        assert off == SP
        cores.append(dict(idxA=wrap16(idx_stream),
                          a_pad=a_stream.astype(BF16)[None, :]))

    struct = dict(SP=SP, tiles=tiles)
    return cores, struct


def _weights_inputs(inputs):
    out = {}
    for b in (0, 1):
        W1 = np.asarray(inputs[f"b{b}_W1"], np.float32)
        b1 = np.asarray(inputs[f"b{b}_b1"], np.float32)
        W2 = np.asarray(inputs[f"b{b}_W2"], np.float32)
        b2 = np.asarray(inputs[f"b{b}_b2"], np.float32)
        Wo = np.asarray(inputs[f"b{b}_Wo"], np.float32)
        bo = np.asarray(inputs[f"b{b}_bo"], np.float32)
        p = f"k{b}_"
        if b == 1:  # block-0 tables come from the host
            out[p + "w1top_m"] = W1[0:128].astype(BF16)
            out[p + "w1top_l"] = W1[128:129].astype(BF16)
            out[p + "w1mid_m"] = W1[D:D + 128].astype(BF16)
            out[p + "w1mid_l"] = W1[D + 128:D + 129].astype(BF16)
            out[p + "b1col"] = b1[:, None].copy()
        out[p + "w1r"] = W1[2 * D:2 * D + 1].astype(BF16)
        out[p + "w2"] = W2.astype(BF16)
        out[p + "b2col"] = b2[:, None].copy()
        out[p + "wo_m"] = Wo[:, 0:128].astype(BF16)
        out[p + "wo_l"] = Wo[:, 128:129].astype(BF16)
        out[p + "bo_m"] = bo[0:128, None].copy()
        out[p + "bo_l"] = bo[128:129, None].copy()
    out["fw1_m"] = np.asarray(inputs["f_W1"], np.float32)[0:128].copy()
    out["fw1_l"] = np.asarray(inputs["f_W1"], np.float32)[128:129].copy()
    out["fb1col"] = np.asarray(inputs["f_b1"], np.float32)[:, None].copy()
    out["fw2"] = np.asarray(inputs["f_W2"], np.float32).copy()
    out["fb2"] = np.asarray(inputs["f_b2"], np.float32)[:, None].copy()
    out["ident"] = np.eye(128, dtype=BF16)
    return out


# ---------------------------------------------------------------------------
# bass graph
# ---------------------------------------------------------------------------

def build_graph(struct):
    import concourse.bacc as bacc
    import concourse.bass as bass
    import concourse.mybir as mybir
    import concourse.tile as tile
    from contextlib import ExitStack

    f32 = mybir.dt.float32
    bf16 = mybir.dt.bfloat16
    i16 = mybir.dt.int16
    Alu = mybir.AluOpType
    Act = mybir.ActivationFunctionType

    SP = struct["SP"]
    tiles = struct["tiles"]
    NT = len(tiles)
    NPCP = NPC + 64  # padded bT width for strided-view headroom

    nc = bacc.Bacc("TRN2", target_bir_lowering=False, debug=False,
                   num_devices=NCORES, num_swdge_queues=NQ)

    din = {}
    def dparam(name, shape, dtype):
        din[name] = nc.dram_tensor(name, list(shape), dtype, kind="ExternalInput")
        return din[name]

    dparam("idxA", (128, SP // 16), i16)
    dparam("a_pad", (1, SP), bf16)
    dparam("interpT", (D, NPC), f32)
    dparam("interpTb", (D, NPC), bf16)
    dparam("A0tab", (N, H), bf16)
    dparam("B0T", (128, NPC), bf16)
    shapes = dict(w1top_m=(128, 128), w1top_l=(1, 128), w1mid_m=(128, 128),
                  w1mid_l=(1, 128), w1r=(1, 128), w2=(128, 128),
                  wo_m=(128, 128), wo_l=(128, 1), b1col=(128, 1),
                  b2col=(128, 1), bo_m=(128, 1), bo_l=(1, 1))
    wnames_bf = ["w1r", "w2", "wo_m", "wo_l"]
    wnames_f32 = ["b2col", "bo_m", "bo_l"]
    for b in (0, 1):
        for w in wnames_bf:
            dparam(f"k{b}_{w}", shapes[w], bf16)
        for w in wnames_f32:
            dparam(f"k{b}_{w}", shapes[w], f32)
    for w in ("w1top_m", "w1top_l", "w1mid_m", "w1mid_l"):
        dparam(f"k1_{w}", shapes[w], bf16)
    dparam("k1_b1col", shapes["b1col"], f32)
    dparam("ident", (128, 128), bf16)
    dparam("fw1_m", (128, 64), f32)
    dparam("fw1_l", (1, 64), f32)
    dparam("fb1col", (64, 1), f32)
    dparam("fw2", (64, 1), f32)
    dparam("fb2", (1, 1), f32)
    out_dram = nc.dram_tensor("out", [1, NPC], f32, kind="ExternalOutput")

    with tile.TileContext(nc) as tc, ExitStack() as ctx:
        per = ctx.enter_context(tc.tile_pool(name="per", bufs=1))
        work = ctx.enter_context(tc.tile_pool(name="work", bufs=2))
        ps = ctx.enter_context(tc.tile_pool(name="ps", bufs=4, space="PSUM"))
        dram = ctx.enter_context(tc.tile_pool(name="dram", bufs=1, space="DRAM"))

        idxA = per.tile([128, SP // 16], i16, tag="idxA", name="idxA")
        nc.sync.dma_start(idxA[:], din["idxA"][:])

        im = [per.tile([128, NPC], f32, tag=f"im{i}", name=f"im{i}") for i in range(2)]
        il = [per.tile([1, NPC], f32, tag=f"il{i}", name=f"il{i}") for i in range(2)]
        imb = [per.tile([128, NPC], bf16, tag=f"imb{i}", name=f"imb{i}") for i in range(2)]
        ilb = [per.tile([1, NPC], bf16, tag=f"ilb{i}", name=f"ilb{i}") for i in range(2)]
        nc.sync.dma_start(im[0][:], din["interpT"][0:128, :])
        nc.sync.dma_start(il[0][:], din["interpT"][128:129, :])
        nc.sync.dma_start(imb[0][:], din["interpTb"][0:128, :])
        nc.sync.dma_start(ilb[0][:], din["interpTb"][128:129, :])

        wsb = {}
        for name, t in din.items():
            if name in ("idxA", "a_pad", "interpT", "interpTb", "A0tab",
                        "B0T"):
                continue
            shp = list(t.shape)
            wsb[name] = per.tile(shp, t.dtype, tag=name, name=name)
            nc.scalar.dma_start(wsb[name][:], t[:])

        pooled = per.tile([128, NPCP], bf16, tag="pooled", name="pooled")
        pooled2 = per.tile([128, NPCP], bf16, tag="pooled2", name="pooled2")
        bT = per.tile([128, NPCP], bf16, tag="bT", name="bT")
        nc.sync.dma_start(bT[:, :NPC], din["B0T"][:])

        warm_own = dram.tile([32, 32], bf16, name="warm_own")
        warm_full = dram.tile([256, 32], bf16, name="warm_full",
                              addr_space="Shared")
        nc.vector.memset(warm_src := per.tile([32, 32], bf16, tag="warm",
                                              name="warm"), 0.0)
        nc.sync.dma_start(warm_own[:], warm_src[:])
        nc.gpsimd.collective_compute(
            "AllGather", Alu.bypass,
            replica_groups=[list(range(NCORES))],
            ins=[warm_own[:].opt()], outs=[warm_full[:].opt()])

        tab_own1 = dram.tile([NPC, H], bf16, name="tab_own1")
        tab_full1 = dram.tile([N, H], bf16, name="tab_full1",
                              addr_space="Shared")
        SGT = 496

        # ---- gather machinery: prepare_only + trigger over NQ queues ----
        qsem = [nc.alloc_semaphore(f"gq{q}") for q in range(NQ)]
        warmsem = nc.alloc_semaphore("gwarm")
        for s in qsem + [warmsem]:
            nc.gpsimd.sem_clear(s)
        qcount = [0] * NQ
        last_pe_mm = [None]

        # dummy 128-idx gather to pull in the gather ucode library while the
        # input DMAs stream; otherwise the first real prep pays ~13us
        warmidx = per.tile([128, 8], i16, tag="warmidx", name="warmidx")
        nc.gpsimd.memset(warmidx[:], 0)
        warmdst = per.tile([128, 128], bf16, tag="warmdst", name="warmdst")
        nc.gpsimd.dma_gather(
            warmdst[:].unsqueeze(1), din["A0tab"][:], warmidx[:],
            128, 128, H, transpose=False, single_packet=False,
            queue_num=0, prepare_only=True, sem=warmsem)
        nc.gpsimd.trigger_dma(count=None, queue_num=0)

        def gather_preps(blk, tis, offs):
            """Issue a wave of preps (one per queue, desc-gen runs
            concurrently on separate Q7 cores).
            Returns [(gA tile, queue, wait-count)] per tile."""
            src = din["A0tab"] if blk == 0 else tab_full1
            out = []
            for ti in tis:
                W = tiles[ti]["width"]
                off = offs[ti]
                q = ti % NQ
                gA = work.tile([128, TILE_SLOTS], bf16, tag="gA", name="gA",
                               bufs=NT)
                nc.gpsimd.dma_gather(
                    gA[:, :W].rearrange("p (c e) -> p c e", e=H),
                    src[:], idxA[:, off // 16:(off + W) // 16],
                    W, W, H, transpose=False, single_packet=False,
                    queue_num=q, prepare_only=True, sem=qsem[q])
                qcount[q] += 1
                out.append((gA, q, qcount[q]))
            return out

        def gather_triggers(tis):
            for q in sorted({ti % NQ for ti in tis}):
                nc.gpsimd.trigger_dma(count=None, queue_num=q)

        def gather_wave(blk, tis, offs):
            out = gather_preps(blk, tis, offs)
            gather_triggers(tis)
            return out

        NTT = 124  # 1984 = 16*124

        def build_tables(tt_range):
            """Block-1 A rows (node-major -> DRAM) and B' (feat-major ->
            SBUF bT), from the updated features; runs on ACT."""
            curb_m, curb_l = imb[1], ilb[1]
            kw = lambda w: wsb[f"k1_{w}"]
            for tt in tt_range:
                sl = slice(tt * NTT, (tt + 1) * NTT)
                psA = ps.tile([NTT, H], f32, tag="psx", name="psx", bufs=2)
                nc.tensor.matmul(psA[:], curb_m[:, sl], kw("w1top_m")[:],
                                 start=True, stop=False)
                nc.tensor.matmul(psA[:], curb_l[:, sl], kw("w1top_l")[:],
                                 start=False, stop=True)
                rA = work.tile([NTT, H], bf16, tag="rowA", name="rowA")
                nc.scalar.copy(rA[:], psA[:])
                nc.sync.dma_start(tab_own1[sl, :], rA[:])

                psB = ps.tile([H, NTT], f32, tag="psh", name="psh", bufs=3)
                nc.tensor.matmul(psB[:], kw("w1mid_m")[:], curb_m[:, sl],
                                 start=True, stop=False)
                nc.tensor.matmul(psB[:], kw("w1mid_l")[:], curb_l[:, sl],
                                 start=False, stop=True)
                nc.scalar.activation(bT[:, sl], psB[:], Act.Identity,
                                     bias=kw("b1col")[:])

        def allgather1():
            nc.gpsimd.collective_compute(
                "AllGather", Alu.bypass,
                replica_groups=[list(range(NCORES))],
                ins=[tab_own1[:].opt()],
                outs=[tab_full1[:].opt()],
            )

        # SGT defined above; 1984 = 4*496

        def blockout_segtile(blk, st):
            kw = lambda w: wsb[f"k{blk}_{w}"]
            cur_m, cur_l = im[blk % 2], il[blk % 2]
            nxt_m, nxt_l = im[(blk + 1) % 2], il[(blk + 1) % 2]
            nxtb_m, nxtb_l = imb[(blk + 1) % 2], ilb[(blk + 1) % 2]
            sl = slice(st * SGT, (st + 1) * SGT)
            # pooled2 = relu(pooled + b2) (relu+bias commuted past the max)
            nc.scalar.activation(pooled2[:, sl], pooled[:, sl], Act.Relu,
                                 bias=kw("b2col")[:])
            po1 = ps.tile([128, SGT], f32, tag="psx", name="psx", bufs=2)
            nc.tensor.matmul(po1[:], kw("wo_m")[:], pooled2[:, sl],
                             start=True, stop=True)
            nc.vector.scalar_tensor_tensor(
                nxt_m[:, sl], po1[:], kw("bo_m")[:], cur_m[:, sl],
                op0=Alu.add, op1=Alu.add)
            if blk == 0:
                nc.scalar.copy(nxtb_m[:, sl], nxt_m[:, sl])
            po2 = ps.tile([1, SGT], f32, tag="psh", name="psh", bufs=3)
            nc.tensor.matmul(po2[:], kw("wo_l")[:], pooled2[:, sl],
                             start=True, stop=True)
            nc.vector.scalar_tensor_tensor(
                nxt_l[:, sl], po2[:], kw("bo_l")[:], cur_l[:, sl],
                op0=Alu.add, op1=Alu.add)
            if blk == 0:
                nc.scalar.copy(nxtb_l[:, sl], nxt_l[:, sl])

        def bview(node0, nstride, nseg, s):
            """bT[:, node0::nstride][:nseg], broadcast to (128, nseg, s)."""
            v = bT[:, node0:node0 + nseg * nstride]
            v = v.rearrange("p (n k) -> p n k", k=nstride)[:, :, 0:1]
            return v.broadcast_to([128, nseg, s])

        def tile_phase1(blk, t, off, gA, q, cnt):
            """Gather-wait, transpose+w1r into PSUM, t1 = pst + B'."""
            kw = lambda w: wsb[f"k{blk}_{w}"]
            W = t["width"]
            a_sb = work.tile([1, TILE_SLOTS], bf16, tag="a_sb",
                             name="a_sb", bufs=3)
            nc.sync.dma_start(a_sb[:, :W], din["a_pad"][:, off:off + W])
            wi = nc.tensor.wait_ge(qsem[q], 16 * cnt)
            if last_pe_mm[0] is not None:
                # order-only dep: stop the scheduler from hoisting this wait
                # ahead of earlier tiles' PE work (PE would stall on a later
                # tile's gather while earlier tiles' data sits ready)
                bass._add_dep_helper(wi.ins, last_pe_mm[0].ins, sync=False,
                                     reason="pin gather-wait after prev tile")
            t1 = work.tile([128, TILE_SLOTS], bf16, tag="t1", name="t1",
                           bufs=3)
            gAv = gA[:].rearrange("p (c e) -> p c e", e=H)
            chunk_pieces = {}
            for pc in t["pieces"]:
                chunk_pieces.setdefault(pc[4] // 512, []).append(pc)
            for c0 in range(0, W, 512):
                w = min(512, W - c0)
                sl = slice(c0, c0 + w)
                pst = ps.tile([128, 512], f32, tag="pst", name="pst",
                              bufs=3)
                # pst = w1r (x) a  +  gA^T   (accumulated in PSUM)
                nc.tensor.matmul(pst[:, :w], kw("w1r")[:], a_sb[:, sl],
                                 start=True, stop=False,
                                 skip_group_check=True)
                nk = w // 128
                for k in range(nk):
                    mm = nc.tensor.matmul(
                        pst[:, k * 128:(k + 1) * 128],
                        gAv[:, c0 // 128 + k, :], wsb["ident"][:],
                        start=False, stop=True, skip_group_check=True)
                    # order-only edge: gA readers must not hoist above the
                    # gather-completion wait (Tile does not auto-gate
                    # prepare_only gather dst reads)
                    bass._add_dep_helper(mm.ins, wi.ins, sync=False,
                                         reason="transpose after gather wait")
                # t1 = pst + B'[seg] per piece
                for (s, nseg, node0, nstride, sloff) in chunk_pieces.get(
                        c0 // 512, ()):
                    pv = pst[:, sloff - c0:sloff - c0 + nseg * s].rearrange(
                        "p (n k) -> p n k", k=s)
                    tv = t1[:, sloff:sloff + nseg * s].rearrange(
                        "p (n k) -> p n k", k=s)
                    nc.vector.scalar_tensor_tensor(
                        tv, pv, 1.0, bview(node0, nstride, nseg, s),
                        op0=Alu.mult, op1=Alu.add)
            return (blk, t, t1, chunk_pieces)

        def tile_phase2(state):
            """relu -> W2 -> ragged segment max for a phase1-completed tile."""
            blk, t, t1, chunk_pieces = state
            kw = lambda w: wsb[f"k{blk}_{w}"]
            W = t["width"]
            h1 = work.tile([128, TILE_SLOTS], bf16, tag="h1", name="h1",
                           bufs=3)
            for c0 in range(0, W, 512):
                w = min(512, W - c0)
                sl = slice(c0, c0 + w)
                nc.scalar.activation(h1[:, sl], t1[:, sl], Act.Relu)
                psh = ps.tile([128, 512], f32, tag="psh", name="psh",
                              bufs=3)
                last_pe_mm[0] = nc.tensor.matmul(psh[:, :w], kw("w2")[:],
                                                 h1[:, sl],
                                                 start=True, stop=True)
                # ragged segment max straight from PSUM (pre-relu/bias)
                for (s, nseg, node0, nstride, sloff) in chunk_pieces.get(
                        c0 // 512, ()):
                    src = psh[:, sloff - c0:sloff - c0 + nseg * s].rearrange(
                        "p (n k) -> p n k", k=s)
                    dst = pooled[:, node0:node0 + nseg * nstride].rearrange(
                        "p (n k) -> p n k", k=nstride)[:, :, 0]
                    nc.vector.tensor_reduce(dst, src,
                                            axis=mybir.AxisListType.X,
                                            op=Alu.max)

        def final_segtile(st):
            fin_m, fin_l = im[0], il[0]
            sl = slice(st * SGT, (st + 1) * SGT)
            pz1 = ps.tile([64, SGT], f32, tag="psx", name="psx", bufs=2)
            nc.tensor.matmul(pz1[:], wsb["fw1_m"][:], fin_m[:, sl],
                             start=True, stop=False)
            nc.tensor.matmul(pz1[:], wsb["fw1_l"][:], fin_l[:, sl],
                             start=False, stop=True)
            z1 = work.tile([64, SGT], f32, tag="z1", name="z1")
            nc.scalar.activation(z1[:], pz1[:], Act.Relu,
                                 bias=wsb["fb1col"][:])
            pz2 = ps.tile([1, SGT], f32, tag="psh", name="psh", bufs=3)
            nc.tensor.matmul(pz2[:], wsb["fw2"][:], z1[:],
                             start=True, stop=True)
            osb = work.tile([1, SGT], f32, tag="osb", name="osb")
            nc.scalar.activation(osb[:], pz2[:], Act.Identity,
                                 bias=wsb["fb2"][:])
            nc.sync.dma_start(out_dram[:, sl], osb[:])

        offs = []
        off = 0
        for t in tiles:
            offs.append(off)
            off += t["width"]

        waves = [list(range(w0, min(w0 + NQ, NT)))
                 for w0 in range(0, NT, NQ)]

        for blk in range(2):
            # all gather waves issued up front (one gA buffer per tile, so
            # no prep ever waits on compute); ring backpressure paces Pool
            handles = {}
            for wave in waves:
                for ti, h in zip(wave, gather_wave(blk, wave, offs)):
                    handles[ti] = h
            for ti, t in enumerate(tiles):
                gA, q, cnt = handles[ti]
                tile_phase2(tile_phase1(blk, t, offs[ti], gA, q, cnt))
            for st in range(NPC // SGT):
                blockout_segtile(blk, st)
                if blk == 0:
                    build_tables(range(st * SGT // NTT,
                                       (st + 1) * SGT // NTT))
                else:
                    final_segtile(st)
            if blk == 0:
                allgather1()

    nc.compile()
    return nc


# ---------------------------------------------------------------------------
# entry point
# ---------------------------------------------------------------------------

def prepare(inputs):
    """Host preprocessing + graph build. Returns (nc, in_maps)."""
    sizes = np.asarray(inputs["neighborhood_sizes"], np.int64)
    nbr = np.asarray(inputs["neighborhoods_indexes"], np.int64)
    a = np.asarray(inputs["add_info"], np.float32)[:, 0]
    interp = np.asarray(inputs["interpolated"], np.float32)

    cores, struct = _preprocess(sizes, nbr, a)
    wmap = _weights_inputs(inputs)

    # block-0 tables on the host
    W1_0 = np.asarray(inputs["b0_W1"], np.float32)
    b1_0 = np.asarray(inputs["b0_b1"], np.float32)
    interp_b = interp.astype(BF16).astype(np.float32)
    A0 = (interp_b @ W1_0[0:D].astype(BF16).astype(np.float32)).astype(BF16)
    B0 = (interp_b @ W1_0[D:2 * D].astype(BF16).astype(np.float32)
          + b1_0).astype(BF16)

    nc = build_graph(struct)

    in_maps = []
    for c in range(NCORES):
        m = dict(wmap)
        m["idxA"] = cores[c]["idxA"]
        m["a_pad"] = cores[c]["a_pad"]
        m["interpT"] = np.ascontiguousarray(
            interp[c * NPC:(c + 1) * NPC].T)
        m["interpTb"] = m["interpT"].astype(BF16)
        m["A0tab"] = A0
        m["B0T"] = np.ascontiguousarray(B0[c * NPC:(c + 1) * NPC].T)
        in_maps.append(m)
    return nc, in_maps


def kernel(**inputs):
    from concourse.bass_utils import run_bass_kernel_spmd

    nc, in_maps = prepare(inputs)
    res = run_bass_kernel_spmd(nc, in_maps, core_ids=list(range(NCORES)))
    out = np.concatenate([res.results[c]["out"].reshape(-1)
                          for c in range(NCORES)])
    return out[:, None].astype(np.float32)


if __name__ == "__main__":
    import jax
    cpu = jax.devices("cpu")[0]
    with jax.default_device(cpu):
        import reference as ref
        inp = ref.setup_inputs()
        expected = np.asarray(ref.reference(**inp))
    inp_np = {k: np.asarray(v) for k, v in inp.items()}
    actual = kernel(**inp_np)
    err = np.linalg.norm(actual - expected) / np.linalg.norm(expected)
    print("Relative error:", err)


# revision 39
# speedup vs baseline: 1.0523x; 1.0523x over previous
"""Trainium2 (8 NeuronCores) kernel for nn_BlockModel_9758165696627.

GNN message passing: 2 residual blocks of
  gather(nbr) + gather(self) + add_info -> MLP(relu) -> segment_max -> @Wo + residual
then a final 129 -> 64 -> 1 MLP.

Strategy (node/segment sharding, v2):
  N = 15872 nodes, E = 253952 edges. Each of the 8 cores owns N/8 = 1984
  contiguous segments; since self_indexes are sorted, each core owns a
  contiguous slice of edges and the segment max is entirely local.

  First-layer refactor: x @ W1 = A[nbr] + B[self] + w1r*add_info where
  A = interp @ W1[:129], B = interp @ W1[129:258] + b1, w1r = W1[258].

  Block 0's A/B tables depend only on kernel inputs, so the host computes
  them (A0 replicated to every core's DRAM, B0' direct to SBUF) - no
  startup AllGather.  Block 1's tables are built on device from the
  updated features and AllGathered as before.

  Per-edge A rows are fetched with NON-transpose dma_gather (256B
  contiguous writes), pipelined with prepare_only+trigger_dma across all
  4 SWDGE queues - concurrent queues give ~2-4x aggregate gather
  bandwidth, and non-transpose mode is safe across queues (transpose
  xbar sprays corrupt each other; verified on HW).  The edge-major tiles
  are transposed to feature-major on the idle PE via identity matmuls
  into PSUM, with the w1r*add_info outer product accumulated into the
  same PSUM bank.

  h2's relu+bias commute past the ragged segment max (relu monotonic,
  bias per-feature), so the max reduces W2's PSUM output directly and
  relu(x+b2) is applied to the pooled (N-sized) result only.

  Edges are laid out region-major, padded so no segment straddles a
  512-slot PSUM-bank boundary.
"""

import numpy as np
import ml_dtypes

BF16 = ml_dtypes.bfloat16

N = 15872
D = 129
H = 128
NCORES = 8
NPC = N // NCORES  # 1984 nodes per core
TILE_SLOTS = 2048  # edge slots per gather tile (4 x 512)
NQ = 4             # SWDGE queues


# ---------------------------------------------------------------------------
# host-side preprocessing
# ---------------------------------------------------------------------------

def _arith_runs(nodes):
    """Split a sorted node list into (start, stride, count) arithmetic runs."""
    runs = []
    i = 0
    n = len(nodes)
    while i < n:
        if i + 1 == n:
            runs.append((int(nodes[i]), 1, 1))
            break
        stride = int(nodes[i + 1] - nodes[i])
        j = i + 1
        while j + 1 < n and nodes[j + 1] - nodes[j] == stride:
            j += 1
        runs.append((int(nodes[i]), stride, j - i + 1))
        i = j + 1
    return runs


def _layout(sizes_core):
    """Region-major tiles with 512-aligned segments.

    Each tile is dict(width, pieces); pieces are
    (size, nseg, node0, node_stride, slot_off), all segments of a piece
    inside one 512-slot block (padding inserted at block boundaries).
    """
    order = []
    for s in np.unique(sizes_core):
        nodes = np.where(sizes_core == s)[0]
        for (n0, st, cnt) in _arith_runs(nodes):
            order.append((int(s), n0, st, cnt))

    tiles = []
    cur = []
    cur_slots = 0

    def cap():
        # first 4 tiles stay small (512 slots): their transfers land fast,
        # so compute starts early at block start and right after the AG
        return 512 if len(tiles) < 4 else TILE_SLOTS

    def flush():
        nonlocal cur, cur_slots
        if cur:
            w = -(-cur_slots // 128) * 128
            tiles.append(dict(width=w, pieces=cur))
            cur, cur_slots = [], 0

    for (s, n0, st, cnt) in order:
        done = 0
        while done < cnt:
            room = (512 - cur_slots % 512) // s
            if room == 0:
                cur_slots = -(-cur_slots // 512) * 512  # pad to 512
                if cur_slots >= cap():
                    flush()
                room = 512 // s
            take = min(cnt - done, room,
                       (cap() - cur_slots) // s if s else 0)
            cur.append((s, take, n0 + done * st, st, cur_slots))
            cur_slots += take * s
            done += take
            if cur_slots >= cap():
                flush()
    flush()
    tiles.sort(key=lambda t: t["width"])
    return tiles


def _preprocess(sizes, nbr, a):
    csum = np.zeros(N + 1, np.int64)
    np.cumsum(sizes, out=csum[1:])

    sizes0 = sizes[:NPC]
    uniform = all(
        np.array_equal(sizes[c * NPC:(c + 1) * NPC], sizes0)
        for c in range(NCORES)
    )
    assert uniform, "per-core segment-size patterns differ; unsupported"
    tiles = _layout(sizes0)
    SP = sum(t["width"] for t in tiles)

    def wrap16(idx):
        n = len(idx)
        assert n % 16 == 0
        w = idx.astype(np.int16).reshape(n // 16, 16).T
        return np.ascontiguousarray(np.tile(w, (8, 1)))

    cores = []
    for c in range(NCORES):
        idx_stream = np.zeros(SP, np.int64)
        a_stream = np.zeros(SP, np.float32)
        off = 0
        for t in tiles:
            for (s, nseg, node0, nstride, slot_off) in t["pieces"]:
                for k in range(nseg):
                    g = c * NPC + node0 + k * nstride
                    e0, e1 = csum[g], csum[g + 1]
                    assert e1 - e0 == s
                    p = off + slot_off + k * s
                    idx_stream[p:p + s] = nbr[e0:e1]
                    a_stream[p:p + s] = a[e0:e1]
            off += t["width"]
        assert off == SP
        cores.append(dict(idxA=wrap16(idx_stream),
                          a_pad=a_stream.astype(BF16)[None, :]))

    struct = dict(SP=SP, tiles=tiles)
    return cores, struct


def _weights_inputs(inputs):
    out = {}
    for b in (0, 1):
        W1 = np.asarray(inputs[f"b{b}_W1"], np.float32)
        b1 = np.asarray(inputs[f"b{b}_b1"], np.float32)
        W2 = np.asarray(inputs[f"b{b}_W2"], np.float32)
        b2 = np.asarray(inputs[f"b{b}_b2"], np.float32)
        Wo = np.asarray(inputs[f"b{b}_Wo"], np.float32)
        bo = np.asarray(inputs[f"b{b}_bo"], np.float32)
        p = f"k{b}_"
        if b == 1:  # block-0 tables come from the host
            out[p + "w1top_m"] = W1[0:128].astype(BF16)
            out[p + "w1top_l"] = W1[128:129].astype(BF16)
            out[p + "w1mid_m"] = W1[D:D + 128].astype(BF16)
            out[p + "w1mid_l"] = W1[D + 128:D + 129].astype(BF16)
            out[p + "b1col"] = b1[:, None].copy()
        out[p + "w1r"] = W1[2 * D:2 * D + 1].astype(BF16)
        out[p + "w2"] = W2.astype(BF16)
        out[p + "b2col"] = b2[:, None].copy()
        out[p + "wo_m"] = Wo[:, 0:128].astype(BF16)
        out[p + "wo_l"] = Wo[:, 128:129].astype(BF16)
        out[p + "bo_m"] = bo[0:128, None].copy()
        out[p + "bo_l"] = bo[128:129, None].copy()
    out["fw1_m"] = np.asarray(inputs["f_W1"], np.float32)[0:128].copy()
    out["fw1_l"] = np.asarray(inputs["f_W1"], np.float32)[128:129].copy()
    out["fb1col"] = np.asarray(inputs["f_b1"], np.float32)[:, None].copy()
    out["fw2"] = np.asarray(inputs["f_W2"], np.float32).copy()
    out["fb2"] = np.asarray(inputs["f_b2"], np.float32)[:, None].copy()
    out["ident"] = np.eye(128, dtype=BF16)
    return out


# ---------------------------------------------------------------------------
# bass graph
# ---------------------------------------------------------------------------

def build_graph(struct):
    import concourse.bacc as bacc
    import concourse.bass as bass
    import concourse.mybir as mybir
    import concourse.tile as tile
    from contextlib import ExitStack

    f32 = mybir.dt.float32
    bf16 = mybir.dt.bfloat16
    i16 = mybir.dt.int16
    Alu = mybir.AluOpType
    Act = mybir.ActivationFunctionType

    SP = struct["SP"]
    tiles = struct["tiles"]
    NT = len(tiles)
    NPCP = NPC + 64  # padded bT width for strided-view headroom

    nc = bacc.Bacc("TRN2", target_bir_lowering=False, debug=False,
                   num_devices=NCORES, num_swdge_queues=NQ)

    din = {}
    def dparam(name, shape, dtype):
        din[name] = nc.dram_tensor(name, list(shape), dtype, kind="ExternalInput")
        return din[name]

    dparam("idxA", (128, SP // 16), i16)
    dparam("a_pad", (1, SP), bf16)
    dparam("interpT", (D, NPC), f32)
    dparam("interpTb", (D, NPC), bf16)
    dparam("A0tab", (N, H), bf16)
    dparam("B0T", (128, NPC), bf16)
    shapes = dict(w1top_m=(128, 128), w1top_l=(1, 128), w1mid_m=(128, 128),
                  w1mid_l=(1, 128), w1r=(1, 128), w2=(128, 128),
                  wo_m=(128, 128), wo_l=(128, 1), b1col=(128, 1),
                  b2col=(128, 1), bo_m=(128, 1), bo_l=(1, 1))
    wnames_bf = ["w1r", "w2", "wo_m", "wo_l"]
    wnames_f32 = ["b2col", "bo_m", "bo_l"]
    for b in (0, 1):
        for w in wnames_bf:
            dparam(f"k{b}_{w}", shapes[w], bf16)
        for w in wnames_f32:
            dparam(f"k{b}_{w}", shapes[w], f32)
    for w in ("w1top_m", "w1top_l", "w1mid_m", "w1mid_l"):
        dparam(f"k1_{w}", shapes[w], bf16)
    dparam("k1_b1col", shapes["b1col"], f32)
    dparam("ident", (128, 128), bf16)
    dparam("fw1_m", (128, 64), f32)
    dparam("fw1_l", (1, 64), f32)
    dparam("fb1col", (64, 1), f32)
    dparam("fw2", (64, 1), f32)
    dparam("fb2", (1, 1), f32)
    out_dram = nc.dram_tensor("out", [1, NPC], f32, kind="ExternalOutput")

    with tile.TileContext(nc) as tc, ExitStack() as ctx:
        per = ctx.enter_context(tc.tile_pool(name="per", bufs=1))
        work = ctx.enter_context(tc.tile_pool(name="work", bufs=2))
        ps = ctx.enter_context(tc.tile_pool(name="ps", bufs=4, space="PSUM"))
        dram = ctx.enter_context(tc.tile_pool(name="dram", bufs=1, space="DRAM"))

        idxA = per.tile([128, SP // 16], i16, tag="idxA", name="idxA")
        nc.sync.dma_start(idxA[:], din["idxA"][:])

        warm_own = dram.tile([32, 32], bf16, name="warm_own")
        warm_full = dram.tile([256, 32], bf16, name="warm_full",
                              addr_space="Shared")
        nc.vector.memset(warm_src := per.tile([32, 32], bf16, tag="warm",
                                              name="warm"), 0.0)
        nc.sync.dma_start(warm_own[:], warm_src[:])
        nc.gpsimd.collective_compute(
            "AllGather", Alu.bypass,
            replica_groups=[list(range(NCORES))],
            ins=[warm_own[:].opt()], outs=[warm_full[:].opt()])

        tab_own1 = dram.tile([NPC, H], bf16, name="tab_own1")
        tab_full1 = dram.tile([N, H], bf16, name="tab_full1",
                              addr_space="Shared")
        SGT = 496

        # ---- gather machinery: prepare_only + trigger over NQ queues ----
        qsem = [nc.alloc_semaphore(f"gq{q}") for q in range(NQ)]
        for s in qsem:
            nc.gpsimd.sem_clear(s)
        qcount = [0] * NQ
        last_pe_mm = [None]

        def gather_preps(blk, tis, offs):
            """Issue a wave of preps (one per queue, desc-gen runs
            concurrently on separate Q7 cores).
            Returns [(gA tile, queue, wait-count)] per tile."""
            src = din["A0tab"] if blk == 0 else tab_full1
            out = []
            for ti in tis:
                W = tiles[ti]["width"]
                off = offs[ti]
                q = ti % NQ
                gA = work.tile([128, TILE_SLOTS], bf16, tag="gA", name="gA",
                               bufs=NT)
                nc.gpsimd.dma_gather(
                    gA[:, :W].rearrange("p (c e) -> p c e", e=H),
                    src[:], idxA[:, off // 16:(off + W) // 16],
                    W, W, H, transpose=False, single_packet=False,
                    queue_num=q, prepare_only=True, sem=qsem[q])
                qcount[q] += 1
                out.append((gA, q, qcount[q]))
            return out

        def gather_triggers(tis):
            for q in sorted({ti % NQ for ti in tis}):
                nc.gpsimd.trigger_dma(count=None, queue_num=q)

        def gather_wave(blk, tis, offs):
            out = gather_preps(blk, tis, offs)
            gather_triggers(tis)
            return out

        NTT = 124  # 1984 = 16*124

        def build_tables(tt_range):
            """Block-1 A rows (node-major -> DRAM) and B' (feat-major ->
            SBUF bT), from the updated features; runs on ACT."""
            curb_m, curb_l = imb[1], ilb[1]
            kw = lambda w: wsb[f"k1_{w}"]
            for tt in tt_range:
                sl = slice(tt * NTT, (tt + 1) * NTT)
                psA = ps.tile([NTT, H], f32, tag="psx", name="psx", bufs=2)
                nc.tensor.matmul(psA[:], curb_m[:, sl], kw("w1top_m")[:],
                                 start=True, stop=False)
                nc.tensor.matmul(psA[:], curb_l[:, sl], kw("w1top_l")[:],
                                 start=False, stop=True)
                rA = work.tile([NTT, H], bf16, tag="rowA", name="rowA")
                nc.scalar.copy(rA[:], psA[:])
                nc.sync.dma_start(tab_own1[sl, :], rA[:])

                psB = ps.tile([H, NTT], f32, tag="psh", name="psh", bufs=3)
                nc.tensor.matmul(psB[:], kw("w1mid_m")[:], curb_m[:, sl],
                                 start=True, stop=False)
                nc.tensor.matmul(psB[:], kw("w1mid_l")[:], curb_l[:, sl],
                                 start=False, stop=True)
                nc.scalar.activation(bT[:, sl], psB[:], Act.Identity,
                                     bias=kw("b1col")[:])

        def allgather1():
            nc.gpsimd.collective_compute(
                "AllGather", Alu.bypass,
                replica_groups=[list(range(NCORES))],
                ins=[tab_own1[:].opt()],
                outs=[tab_full1[:].opt()],
            )

        # SGT defined above; 1984 = 4*496

        def blockout_segtile(blk, st):
            kw = lambda w: wsb[f"k{blk}_{w}"]
            cur_m, cur_l = im[blk % 2], il[blk % 2]
            nxt_m, nxt_l = im[(blk + 1) % 2], il[(blk + 1) % 2]
            nxtb_m, nxtb_l = imb[(blk + 1) % 2], ilb[(blk + 1) % 2]
            sl = slice(st * SGT, (st + 1) * SGT)
            # pooled2 = relu(pooled + b2) (relu+bias commuted past the max)
            nc.scalar.activation(pooled2[:, sl], pooled[:, sl], Act.Relu,
                                 bias=kw("b2col")[:])
            po1 = ps.tile([128, SGT], f32, tag="psx", name="psx", bufs=2)
            nc.tensor.matmul(po1[:], kw("wo_m")[:], pooled2[:, sl],
                             start=True, stop=True)
            nc.vector.scalar_tensor_tensor(
                nxt_m[:, sl], po1[:], kw("bo_m")[:], cur_m[:, sl],
                op0=Alu.add, op1=Alu.add)
            if blk == 0:
                nc.scalar.copy(nxtb_m[:, sl], nxt_m[:, sl])
            po2 = ps.tile([1, SGT], f32, tag="psh", name="psh", bufs=3)
            nc.tensor.matmul(po2[:], kw("wo_l")[:], pooled2[:, sl],
                             start=True, stop=True)
            nc.vector.scalar_tensor_tensor(
                nxt_l[:, sl], po2[:], kw("bo_l")[:], cur_l[:, sl],
                op0=Alu.add, op1=Alu.add)
            if blk == 0:
                nc.scalar.copy(nxtb_l[:, sl], nxt_l[:, sl])

        def bview(node0, nstride, nseg, s):
            """bT[:, node0::nstride][:nseg], broadcast to (128, nseg, s)."""
            v = bT[:, node0:node0 + nseg * nstride]
            v = v.rearrange("p (n k) -> p n k", k=nstride)[:, :, 0:1]
            return v.broadcast_to([128, nseg, s])

        def tile_phase1(blk, t, off, gA, q, cnt):
            """Gather-wait, transpose+w1r into PSUM, t1 = pst + B'."""
            kw = lambda w: wsb[f"k{blk}_{w}"]
            W = t["width"]
            a_sb = work.tile([1, TILE_SLOTS], bf16, tag="a_sb",
                             name="a_sb", bufs=3)
            nc.sync.dma_start(a_sb[:, :W], din["a_pad"][:, off:off + W])
            wi = nc.tensor.wait_ge(qsem[q], 16 * cnt)
            if last_pe_mm[0] is not None:
                # order-only dep: stop the scheduler from hoisting this wait
                # ahead of earlier tiles' PE work (PE would stall on a later
                # tile's gather while earlier tiles' data sits ready)
                bass._add_dep_helper(wi.ins, last_pe_mm[0].ins, sync=False,
                                     reason="pin gather-wait after prev tile")
            t1 = work.tile([128, TILE_SLOTS], bf16, tag="t1", name="t1",
                           bufs=3)
            gAv = gA[:].rearrange("p (c e) -> p c e", e=H)
            chunk_pieces = {}
            for pc in t["pieces"]:
                chunk_pieces.setdefault(pc[4] // 512, []).append(pc)
            for c0 in range(0, W, 512):
                w = min(512, W - c0)
                sl = slice(c0, c0 + w)
                pst = ps.tile([128, 512], f32, tag="pst", name="pst",
                              bufs=3)
                # pst = w1r (x) a  +  gA^T   (accumulated in PSUM)
                nc.tensor.matmul(pst[:, :w], kw("w1r")[:], a_sb[:, sl],
                                 start=True, stop=False,
                                 skip_group_check=True)
                nk = w // 128
                for k in range(nk):
                    mm = nc.tensor.matmul(
                        pst[:, k * 128:(k + 1) * 128],
                        gAv[:, c0 // 128 + k, :], wsb["ident"][:],
                        start=False, stop=True, skip_group_check=True)
                    # order-only edge: gA readers must not hoist above the
                    # gather-completion wait (Tile does not auto-gate
                    # prepare_only gather dst reads)
                    bass._add_dep_helper(mm.ins, wi.ins, sync=False,
                                         reason="transpose after gather wait")
                # t1 = pst + B'[seg] per piece
                for (s, nseg, node0, nstride, sloff) in chunk_pieces.get(
                        c0 // 512, ()):
                    pv = pst[:, sloff - c0:sloff - c0 + nseg * s].rearrange(
                        "p (n k) -> p n k", k=s)
                    tv = t1[:, sloff:sloff + nseg * s].rearrange(
                        "p (n k) -> p n k", k=s)
                    nc.vector.scalar_tensor_tensor(
                        tv, pv, 1.0, bview(node0, nstride, nseg, s),
                        op0=Alu.mult, op1=Alu.add)
            return (blk, t, t1, chunk_pieces)

        def tile_phase2(state):
            """relu -> W2 -> ragged segment max for a phase1-completed tile."""
            blk, t, t1, chunk_pieces = state
            kw = lambda w: wsb[f"k{blk}_{w}"]
            W = t["width"]
            h1 = work.tile([128, TILE_SLOTS], bf16, tag="h1", name="h1",
                           bufs=3)
            for c0 in range(0, W, 512):
                w = min(512, W - c0)
                sl = slice(c0, c0 + w)
                nc.scalar.activation(h1[:, sl], t1[:, sl], Act.Relu)
                psh = ps.tile([128, 512], f32, tag="psh", name="psh",
                              bufs=3)
                last_pe_mm[0] = nc.tensor.matmul(psh[:, :w], kw("w2")[:],
                                                 h1[:, sl],
                                                 start=True, stop=True)
                # ragged segment max straight from PSUM (pre-relu/bias)
                for (s, nseg, node0, nstride, sloff) in chunk_pieces.get(
                        c0 // 512, ()):
                    src = psh[:, sloff - c0:sloff - c0 + nseg * s].rearrange(
                        "p (n k) -> p n k", k=s)
                    dst = pooled[:, node0:node0 + nseg * nstride].rearrange(
                        "p (n k) -> p n k", k=nstride)[:, :, 0]
                    nc.vector.tensor_reduce(dst, src,
                                            axis=mybir.AxisListType.X,
                                            op=Alu.max)

        def final_segtile(st):
            fin_m, fin_l = im[0], il[0]
            sl = slice(st * SGT, (st + 1) * SGT)
            pz1 = ps.tile([64, SGT], f32, tag="psx", name="psx", bufs=2)
            nc.tensor.matmul(pz1[:], wsb["fw1_m"][:], fin_m[:, sl],
                             start=True, stop=False)
            nc.tensor.matmul(pz1[:], wsb["fw1_l"][:], fin_l[:, sl],
                             start=False, stop=True)
            z1 = work.tile([64, SGT], f32, tag="z1", name="z1")
            nc.scalar.activation(z1[:], pz1[:], Act.Relu,
                                 bias=wsb["fb1col"][:])
            pz2 = ps.tile([1, SGT], f32, tag="psh", name="psh", bufs=3)
            nc.tensor.matmul(pz2[:], wsb["fw2"][:], z1[:],
                             start=True, stop=True)
            osb = work.tile([1, SGT], f32, tag="osb", name="osb")
            nc.scalar.activation(osb[:], pz2[:], Act.Identity,
                                 bias=wsb["fb2"][:])
            nc.sync.dma_start(out_dram[:, sl], osb[:])

        offs = []
        off = 0
        for t in tiles:
            offs.append(off)
            off += t["width"]

        waves = [list(range(w0, min(w0 + NQ, NT)))
                 for w0 in range(0, NT, NQ)]

        # block-0 wave 0 issues before every other input load: the first
        # gathers only need idxA, and the weight-load storm otherwise gates
        # Pool's gather bookkeeping (DMAHW lane waits) until ~23us
        pre0 = {ti: h for ti, h in
                zip(waves[0], gather_wave(0, waves[0], offs))}

        im = [per.tile([128, NPC], f32, tag=f"im{i}", name=f"im{i}") for i in range(2)]
        il = [per.tile([1, NPC], f32, tag=f"il{i}", name=f"il{i}") for i in range(2)]
        imb = [per.tile([128, NPC], bf16, tag=f"imb{i}", name=f"imb{i}") for i in range(2)]
        ilb = [per.tile([1, NPC], bf16, tag=f"ilb{i}", name=f"ilb{i}") for i in range(2)]
        nc.sync.dma_start(im[0][:], din["interpT"][0:128, :])
        nc.sync.dma_start(il[0][:], din["interpT"][128:129, :])
        nc.sync.dma_start(imb[0][:], din["interpTb"][0:128, :])
        nc.sync.dma_start(ilb[0][:], din["interpTb"][128:129, :])

        wsb = {}
        for name, t in din.items():
            if name in ("idxA", "a_pad", "interpT", "interpTb", "A0tab",
                        "B0T"):
                continue
            shp = list(t.shape)
            wsb[name] = per.tile(shp, t.dtype, tag=name, name=name)
            nc.scalar.dma_start(wsb[name][:], t[:])

        pooled = per.tile([128, NPCP], bf16, tag="pooled", name="pooled")
        pooled2 = per.tile([128, NPCP], bf16, tag="pooled2", name="pooled2")
        bT = per.tile([128, NPCP], bf16, tag="bT", name="bT")
        nc.sync.dma_start(bT[:, :NPC], din["B0T"][:])


        for blk in range(2):
            # all gather waves issued up front (one gA buffer per tile, so
            # no prep ever waits on compute); ring backpressure paces Pool
            handles = dict(pre0) if blk == 0 else {}
            for wi, wave in enumerate(waves):
                if blk == 0 and wi == 0:
                    continue  # pre-issued before the other input loads
                for ti, h in zip(wave, gather_wave(blk, wave, offs)):
                    handles[ti] = h
            for ti, t in enumerate(tiles):
                gA, q, cnt = handles[ti]
                tile_phase2(tile_phase1(blk, t, offs[ti], gA, q, cnt))
            for st in range(NPC // SGT):
                blockout_segtile(blk, st)
                if blk == 0:
                    build_tables(range(st * SGT // NTT,
                                       (st + 1) * SGT // NTT))
                else:
                    final_segtile(st)
            if blk == 0:
                allgather1()

    nc.compile()
    return nc


# ---------------------------------------------------------------------------
# entry point
# ---------------------------------------------------------------------------

def prepare(inputs):
    """Host preprocessing + graph build. Returns (nc, in_maps)."""
    sizes = np.asarray(inputs["neighborhood_sizes"], np.int64)
    nbr = np.asarray(inputs["neighborhoods_indexes"], np.int64)
    a = np.asarray(inputs["add_info"], np.float32)[:, 0]
    interp = np.asarray(inputs["interpolated"], np.float32)

    cores, struct = _preprocess(sizes, nbr, a)
    wmap = _weights_inputs(inputs)

    # block-0 tables on the host
    W1_0 = np.asarray(inputs["b0_W1"], np.float32)
    b1_0 = np.asarray(inputs["b0_b1"], np.float32)
    interp_b = interp.astype(BF16).astype(np.float32)
    A0 = (interp_b @ W1_0[0:D].astype(BF16).astype(np.float32)).astype(BF16)
    B0 = (interp_b @ W1_0[D:2 * D].astype(BF16).astype(np.float32)
          + b1_0).astype(BF16)

    nc = build_graph(struct)

    in_maps = []
    for c in range(NCORES):
        m = dict(wmap)
        m["idxA"] = cores[c]["idxA"]
        m["a_pad"] = cores[c]["a_pad"]
        m["interpT"] = np.ascontiguousarray(
            interp[c * NPC:(c + 1) * NPC].T)
        m["interpTb"] = m["interpT"].astype(BF16)
        m["A0tab"] = A0
        m["B0T"] = np.ascontiguousarray(B0[c * NPC:(c + 1) * NPC].T)
        in_maps.append(m)
    return nc, in_maps


def kernel(**inputs):
    from concourse.bass_utils import run_bass_kernel_spmd

    nc, in_maps = prepare(inputs)
    res = run_bass_kernel_spmd(nc, in_maps, core_ids=list(range(NCORES)))
    out = np.concatenate([res.results[c]["out"].reshape(-1)
                          for c in range(NCORES)])
    return out[:, None].astype(np.float32)


if __name__ == "__main__":
    import jax
    cpu = jax.devices("cpu")[0]
    with jax.default_device(cpu):
        import reference as ref
        inp = ref.setup_inputs()
        expected = np.asarray(ref.reference(**inp))
    inp_np = {k: np.asarray(v) for k, v in inp.items()}
    actual = kernel(**inp_np)
    err = np.linalg.norm(actual - expected) / np.linalg.norm(expected)
    print("Relative error:", err)
